# revision 3
# baseline (speedup 1.0000x reference)
"""Seq2seq LSTM (CoordinatePredictionModel) Trainium2 Bass kernel.

Model: 200-step LSTM encoder over [T=200, B=4096, IN=4], then 30-step
autoregressive LSTM decoder with output projection -> [30, B, OUT=4].

Sharding: pure data-parallel over batch. B=4096 -> 512 per core x 8 cores,
no collectives. Each core runs the full 230-step recurrence.

Layout ("hidden on partitions"): per-step state lives transposed in one
SBUF tile s = [K=105, HB]: rows 0..99 = (2h)^T, rows 100..103 = x^T
(encoder input; zero-weighted in decoder), row 104 = ones. Gate
pre-activations for chunk g come from one matmul
  psum[:, g*HB:(g+1)*HB] = W_g^T.T @ s       (K=105 contraction)
with W_g^T = [W_hh_g.T ; W_ih_g.T ; (b_ih+b_hh)_g] stacked on partitions.

All-tanh gates: sigmoid(x) = (tanh(x/2)+1)/2, with the 1/2 folded into
the packed i/f/o weight columns, so ONE scalar-engine tanh covers all
four gate chunks [100, 4*HB] per step (the scalar engine's per-
instruction overhead was the baseline bottleneck). States are stored
scaled: S = 2c, sh = 2h (the 1/2 folded into weight h-rows), making the
cell update three fused scalar_tensor_tensor ops:
  b  = (th_i + 1) * th_g          (DVE)      = 2 sig(i) tanh(g)
  aa = (th_f + 1) * S             (GpSimd)   = 4 sig(f) c
  S' = 0.5*aa + b                 (DVE)      = 2 c'
  tct = tanh(0.5 * S')            (ACT, free input scale) = tanh(c')
  sh' = (th_o + 1) * tct          (DVE)      = 2 h'

Decoder feedback y_prev = W_y h + b_y is folded into the recurrence:
  W_ih_dec @ y_prev + W_hh_dec @ h = (W_ih_dec W_y + W_hh_dec) @ h + W_ih_dec b_y
(valid from the second decoder step; the first uses y_prev = 0), so y is
only ever computed PSUM -> DRAM and never re-enters SBUF state.

Two batch half-chains (nh=2) are software-pipelined anti-phase so each
engine alternates between chains and dependency latency is hidden.

Only DMA may write non-32-aligned partition bases; all engine writes here
start at partition 0 (x and ones rows are DMA-written).
"""

import numpy as np

import concourse.bass as bass
import concourse.mybir as mybir
from concourse import bacc
from concourse.tile import TileContext
from concourse.tile_rust import add_dep_helper
from concourse.bass_utils import run_bass_kernel_spmd

T, B, IN, OUT, H = 200, 4096, 4, 4, 100
DEC = 30
NCORES = 8
BS = B // NCORES          # 512 batch rows per core
K = H + IN + 1            # 105 = h + x + ones
F32 = mybir.dt.float32

# matmul compute dtype: fp32r is fp32 bits in memory, reduced-precision
# (1 cycle/row) on the PE
MDT = mybir.dt.float32r
EWDT = F32

ALU = mybir.AluOpType


def _pack_weights(W_hh, W_ih, bias):
    """[K=105, 4H] stacked lhsT, gate order [i,f,g,o] (pytorch).

    i/f/o columns scaled by 1/2 (sigmoid-as-tanh), h-rows by a further
    1/2 (state stores 2h)."""
    cs = np.ones((4 * H,), np.float64)
    cs[0:2 * H] = 0.5          # i, f
    cs[3 * H:4 * H] = 0.5      # o
    Wk = np.zeros((K, 4 * H), np.float64)
    Wk[0:H, :] = W_hh.T * cs * 0.5
    if W_ih is not None:
        Wk[H:H + W_ih.shape[1], :] = W_ih.T * cs
    Wk[K - 1, :] = bias * cs
    return Wk.astype(np.float32)


def _build_program(nh=2):
    """nh = number of independent batch half-chains interleaved per step."""
    nc = bacc.Bacc("TRN2", debug=False, num_devices=NCORES)

    x_d = nc.dram_tensor("x", (T, IN, BS), MDT, kind="ExternalInput").ap()
    encw_d = nc.dram_tensor("encw", (K, 4 * H), MDT, kind="ExternalInput").ap()
    dec0w_d = nc.dram_tensor("dec0w", (K, 4 * H), MDT, kind="ExternalInput").ap()
    decfw_d = nc.dram_tensor("decfw", (K, 4 * H), MDT, kind="ExternalInput").ap()
    wy_d = nc.dram_tensor("wy", (K, OUT), MDT, kind="ExternalInput").ap()
    ones_d = nc.dram_tensor("ones", (1, BS), MDT, kind="ExternalInput").ap()
    y_d = nc.dram_tensor("y", (DEC, OUT, BS), F32, kind="ExternalOutput").ap()

    AF = mybir.ActivationFunctionType
    HB = BS // nh

    with TileContext(nc) as tc:
        with (
            tc.tile_pool(name="const", bufs=1) as constp,
            tc.tile_pool(name="state", bufs=1) as statep,
            tc.tile_pool(name="work", bufs=2) as work,
            tc.tile_pool(name="psum", bufs=2, space="PSUM") as psump,
        ):
            encw = constp.tile([K, 4 * H], MDT, tag="encw")
            dec0w = constp.tile([K, 4 * H], MDT, tag="dec0w")
            decfw = constp.tile([K, 4 * H], MDT, tag="decfw")
            wy = constp.tile([K, OUT], MDT, tag="wy")
            nc.sync.dma_start(encw[:], encw_d[:])
            nc.sync.dma_start(dec0w[:], dec0w_d[:])
            nc.sync.dma_start(decfw[:], decfw_d[:])
            nc.sync.dma_start(wy[:], wy_d[:])

            sz = [[statep.tile([K, HB], MDT, tag=f"s{p}{z}", name=f"s{p}{z}")
                   for p in range(2)] for z in range(nh)]
            cz = [statep.tile([H, HB], EWDT, tag=f"c{z}", name=f"c{z}")
                  for z in range(nh)]

            for z in range(nh):
                hb = slice(z * HB, (z + 1) * HB)
                nc.gpsimd.memset(sz[z][0][0:H, :].bitcast(mybir.dt.uint32), 0)
                nc.gpsimd.memset(cz[z][:], 0.0)
                nc.sync.dma_start(sz[z][0][K - 1:K, :], ones_d[0:1, hb])
                nc.sync.dma_start(sz[z][1][K - 1:K, :], ones_d[0:1, hb])
                nc.sync.dma_start(sz[z][0][H:H + IN, :], x_d[0, :, hb])

            def emit_P1(z, t):
                """4 gate matmuls + one all-gates tanh. Returns (pt, th)."""
                W = encw if t < T else (dec0w if t == T else decfw)
                prev = sz[z][t % 2]
                pt = psump.tile([H, 4 * HB], F32, tag=f"pt{z}", name=f"pt{z}")
                for g in range(4):
                    nc.tensor.matmul(pt[:, g * HB:(g + 1) * HB],
                                     W[:, g * H:(g + 1) * H], prev[:],
                                     start=True, stop=True)
                th = work.tile([H, 4 * HB], EWDT, tag=f"th{z}", name=f"th{z}")
                nc.scalar.activation(th[:], pt[:], AF.Tanh)
                return pt, th

            def emit_P2(z, t, pt, th):
                """Cell-state tail for chain z, step t."""
                S = cz[z]
                th_i = th[:, 0:HB]
                th_f = th[:, HB:2 * HB]
                th_g = th[:, 2 * HB:3 * HB]
                th_o = th[:, 3 * HB:4 * HB]
                b = work.tile([H, HB], EWDT, tag=f"b{z}", name=f"b{z}")
                nc.vector.scalar_tensor_tensor(b[:], th_i, 1.0, th_g,
                                               ALU.add, ALU.mult)
                aa = work.tile([H, HB], EWDT, tag=f"aa{z}", name=f"aa{z}")
                nc.vector.scalar_tensor_tensor(aa[:], th_f, 1.0, S[:],
                                               ALU.add, ALU.mult)
                nc.vector.scalar_tensor_tensor(S[:], aa[:], 0.5, b[:],
                                               ALU.mult, ALU.add)
                tct = work.tile([H, HB], EWDT, tag=f"tct{z}", name=f"tct{z}")
                M = work.tile([H, HB], EWDT, tag=f"M{z}", name=f"M{z}")
                with tc.high_priority():
                    nc.scalar.activation(tct[:], S[:], AF.Tanh, scale=0.5)
                    # 2h = (th_o+1)*tct, split so the slow GpSimd engine
                    # absorbs the multiply (it cannot run TensorScalarPtr)
                    nc.gpsimd.tensor_mul(M[:], th_o, tct[:])
                    nxt = sz[z][(t + 1) % 2]
                    nc.vector.tensor_add(nxt[0:H, :], M[:], tct[:])
                hb = slice(z * HB, (z + 1) * HB)
                if t < T - 1:
                    nc.sync.dma_start(nxt[H:H + IN, :], x_d[t + 1, :, hb])
                elif t == T - 1:
                    nc.vector.memset(S[:], 0.0)
                else:
                    d = t - T
                    yp = pt[0:OUT, 3 * HB:4 * HB]
                    nc.tensor.matmul(yp, wy[:], nxt[:], start=True, stop=True)
                    yo = work.tile([OUT, HB], F32, tag=f"yo{z}", name=f"yo{z}")
                    nc.vector.tensor_copy(yo[:], yp)
                    nc.sync.dma_start(y_d[d, :, hb], yo[:])

            # Software pipeline: chain z's P1 interleaves with the other
            # chain's pending tail, anti-phasing the chains on the
            # in-order engines.
            pend = {}
            for t in range(T + DEC):
                for z in range(nh):
                    pt, th = emit_P1(z, t)
                    zo = (z + 1) % nh
                    if zo in pend:
                        emit_P2(**pend.pop(zo))
                    pend[z] = dict(z=z, t=t, pt=pt, th=th)
            for z in list(pend):
                emit_P2(**pend.pop(z))
    nc.finalize()
    return nc


def kernel(inputs, W_ih_enc, W_hh_enc, b_ih_enc, b_hh_enc,
           W_ih_dec, W_hh_dec, b_ih_dec, b_hh_dec, W_y, b_y,
           _trace=False, _perf_out=None):
    inputs = np.asarray(inputs, np.float32)
    f64 = np.float64
    encw = _pack_weights(np.asarray(W_hh_enc, f64), np.asarray(W_ih_enc, f64),
                         np.asarray(b_ih_enc, f64) + np.asarray(b_hh_enc, f64))
    Wihd = np.asarray(W_ih_dec, f64)
    Whhd = np.asarray(W_hh_dec, f64)
    bd = np.asarray(b_ih_dec, f64) + np.asarray(b_hh_dec, f64)
    Wyf = np.asarray(W_y, f64)
    byf = np.asarray(b_y, f64)
    dec0w = _pack_weights(Whhd, None, bd)
    decfw = _pack_weights(Whhd + Wihd @ Wyf, None, bd + Wihd @ byf)
    wyk = np.zeros((K, OUT), np.float32)
    wyk[0:H, :] = Wyf.T.astype(np.float32) * 0.5
    wyk[K - 1, :] = byf.astype(np.float32)
    ones = np.ones((1, BS), np.float32)

    nc = _build_program()

    in_maps = []
    for core in range(NCORES):
        xs = inputs[:, core * BS:(core + 1) * BS, :]         # [T, BS, IN]
        xt = np.ascontiguousarray(xs.transpose(0, 2, 1))     # [T, IN, BS]
        in_maps.append({"x": xt, "encw": encw, "dec0w": dec0w,
                        "decfw": decfw, "wy": wyk, "ones": ones})

    import time as _time
    res = run_bass_kernel_spmd(nc, in_maps, core_ids=list(range(NCORES)),
                               trace=_trace)
    if _perf_out is not None:
        walls = []
        for _ in range(6):
            t0 = _time.time()
            res = run_bass_kernel_spmd(nc, in_maps,
                                       core_ids=list(range(NCORES)),
                                       trace=_trace)
            walls.append(int((_time.time() - t0) * 1e9))
        _perf_out.update(exec_time_ns=res.exec_time_ns, walls_ns=walls,
                         trace=res.instructions_and_trace,
                         profile_json=res.profile_json)
    out = np.empty((DEC, B, OUT), np.float32)
    for core in range(NCORES):
        y = res.results[core]["y"]                           # [DEC, OUT, BS]
        out[:, core * BS:(core + 1) * BS, :] = y.transpose(0, 2, 1)
    return out


# revision 6
# speedup vs baseline: 1.0349x; 1.0349x over previous
"""Seq2seq LSTM (CoordinatePredictionModel) Trainium2 Bass kernel.

Model: 200-step LSTM encoder over [T=200, B=4096, IN=4], then 30-step
autoregressive LSTM decoder with output projection -> [30, B, OUT=4].

Sharding: pure data-parallel over batch. B=4096 -> 512 per core x 8 cores,
no collectives. Each core runs the full 230-step recurrence.

Layout ("hidden on partitions"): per-step state lives transposed in one
SBUF tile s = [K=105, HB]: rows 0..99 = (2h)^T, rows 100..103 = x^T
(encoder input; zero-weighted in decoder), row 104 = ones. Gate
pre-activations for chunk g come from one matmul
  psum[:, g*HB:(g+1)*HB] = W_g^T.T @ s       (K=105 contraction)
with W_g^T = [W_hh_g.T ; W_ih_g.T ; (b_ih+b_hh)_g] stacked on partitions.

All-tanh gates: sigmoid(x) = (tanh(x/2)+1)/2, with the 1/2 folded into
the packed i/f/o weight columns, so ONE scalar-engine tanh covers all
four gate chunks [100, 4*HB] per step (the scalar engine's per-
instruction overhead was the baseline bottleneck). States are stored
scaled: S = 2c, sh = 2h (the 1/2 folded into weight h-rows), making the
cell update three fused scalar_tensor_tensor ops:
  b  = (th_i + 1) * th_g          (DVE)      = 2 sig(i) tanh(g)
  aa = (th_f + 1) * S             (GpSimd)   = 4 sig(f) c
  S' = 0.5*aa + b                 (DVE)      = 2 c'
  tct = tanh(0.5 * S')            (ACT, free input scale) = tanh(c')
  sh' = (th_o + 1) * tct          (DVE)      = 2 h'

Decoder feedback y_prev = W_y h + b_y is folded into the recurrence:
  W_ih_dec @ y_prev + W_hh_dec @ h = (W_ih_dec W_y + W_hh_dec) @ h + W_ih_dec b_y
(valid from the second decoder step; the first uses y_prev = 0), so y is
only ever computed PSUM -> DRAM and never re-enters SBUF state.

Two batch half-chains (nh=2) are software-pipelined anti-phase so each
engine alternates between chains and dependency latency is hidden.

Only DMA may write non-32-aligned partition bases; all engine writes here
start at partition 0 (x and ones rows are DMA-written).
"""

import ml_dtypes
import numpy as np

import concourse.bass as bass
import concourse.mybir as mybir
from concourse import bacc
from concourse.tile import TileContext
from concourse.tile_rust import add_dep_helper
from concourse.bass_utils import run_bass_kernel_spmd

T, B, IN, OUT, H = 200, 4096, 4, 4, 100
DEC = 30
NCORES = 8
BS = B // NCORES          # 512 batch rows per core
K = H + IN + 1            # 105 = h + x + ones
F32 = mybir.dt.float32

# matmul operand dtype: bf16 streams 1 column/cycle on the PE (fp32/fp32r
# measured 4 cycles/col); PSUM accumulation stays fp32. End-to-end rms
# impact measured 8.7e-4 in a numpy bit-accurate simulation.
MDT = mybir.dt.bfloat16
EWDT = F32

ALU = mybir.AluOpType


def _pack_weights(W_hh, W_ih, bias):
    """[K=105, 4H] stacked lhsT, gate order [i,f,g,o] (pytorch).

    i/f/o columns scaled by 1/2 (sigmoid-as-tanh), h-rows by a further
    1/2 (state stores 2h)."""
    cs = np.ones((4 * H,), np.float64)
    cs[0:2 * H] = 0.5          # i, f
    cs[3 * H:4 * H] = 0.5      # o
    Wk = np.zeros((K, 4 * H), np.float64)
    Wk[0:H, :] = W_hh.T * cs * 0.5
    if W_ih is not None:
        Wk[H:H + W_ih.shape[1], :] = W_ih.T * cs
    Wk[K - 1, :] = bias * cs
    return Wk.astype(np.float32)


def _build_program(nh=2):
    """nh = number of independent batch half-chains interleaved per step."""
    nc = bacc.Bacc("TRN2", debug=False, num_devices=NCORES)

    x_d = nc.dram_tensor("x", (T, IN, BS), MDT, kind="ExternalInput").ap()
    encw_d = nc.dram_tensor("encw", (K, 4 * H), MDT, kind="ExternalInput").ap()
    dec0w_d = nc.dram_tensor("dec0w", (K, 4 * H), MDT, kind="ExternalInput").ap()
    decfw_d = nc.dram_tensor("decfw", (K, 4 * H), MDT, kind="ExternalInput").ap()
    wy_d = nc.dram_tensor("wy", (K, OUT), MDT, kind="ExternalInput").ap()
    ones_d = nc.dram_tensor("ones", (1, BS), MDT, kind="ExternalInput").ap()
    y_d = nc.dram_tensor("y", (DEC, OUT, BS), F32, kind="ExternalOutput").ap()

    AF = mybir.ActivationFunctionType
    HB = BS // nh

    with TileContext(nc) as tc:
        with (
            tc.tile_pool(name="const", bufs=1) as constp,
            tc.tile_pool(name="state", bufs=1) as statep,
            tc.tile_pool(name="work", bufs=2) as work,
            tc.tile_pool(name="psum", bufs=2, space="PSUM") as psump,
        ):
            encw = constp.tile([K, 4 * H], MDT, tag="encw")
            dec0w = constp.tile([K, 4 * H], MDT, tag="dec0w")
            decfw = constp.tile([K, 4 * H], MDT, tag="decfw")
            wy = constp.tile([K, OUT], MDT, tag="wy")
            nc.sync.dma_start(encw[:], encw_d[:])
            nc.sync.dma_start(dec0w[:], dec0w_d[:])
            nc.sync.dma_start(decfw[:], decfw_d[:])
            nc.sync.dma_start(wy[:], wy_d[:])

            sz = [[statep.tile([K, HB], MDT, tag=f"s{p}{z}", name=f"s{p}{z}")
                   for p in range(2)] for z in range(nh)]
            cz = [statep.tile([H, HB], EWDT, tag=f"c{z}", name=f"c{z}")
                  for z in range(nh)]

            for z in range(nh):
                hb = slice(z * HB, (z + 1) * HB)
                nc.gpsimd.memset(sz[z][0][0:H, :].bitcast(mybir.dt.uint16), 0)
                nc.gpsimd.memset(cz[z][:], 0.0)
                nc.sync.dma_start(sz[z][0][K - 1:K, :], ones_d[0:1, hb])
                nc.sync.dma_start(sz[z][1][K - 1:K, :], ones_d[0:1, hb])
                nc.sync.dma_start(sz[z][0][H:H + IN, :], x_d[0, :, hb])

            def emit_P1(z, t):
                """4 gate matmuls + one all-gates tanh. Returns (pt, th)."""
                W = encw if t < T else (dec0w if t == T else decfw)
                prev = sz[z][t % 2]
                pt = psump.tile([H, 4 * HB], F32, tag=f"pt{z}", name=f"pt{z}")
                for g in range(4):
                    nc.tensor.matmul(pt[:, g * HB:(g + 1) * HB],
                                     W[:, g * H:(g + 1) * H], prev[:],
                                     start=True, stop=True)
                th = work.tile([H, 4 * HB], EWDT, tag=f"th{z}", name=f"th{z}")
                nc.scalar.activation(th[:], pt[:], AF.Tanh)
                return pt, th

            def emit_P2(z, t, pt, th):
                """Cell-state tail for chain z, step t."""
                S = cz[z]
                th_i = th[:, 0:HB]
                th_f = th[:, HB:2 * HB]
                th_g = th[:, 2 * HB:3 * HB]
                th_o = th[:, 3 * HB:4 * HB]
                b = work.tile([H, HB], EWDT, tag=f"b{z}", name=f"b{z}")
                nc.vector.scalar_tensor_tensor(b[:], th_i, 1.0, th_g,
                                               ALU.add, ALU.mult)
                aa = work.tile([H, HB], EWDT, tag=f"aa{z}", name=f"aa{z}")
                nc.vector.scalar_tensor_tensor(aa[:], th_f, 1.0, S[:],
                                               ALU.add, ALU.mult)
                nc.vector.scalar_tensor_tensor(S[:], aa[:], 0.5, b[:],
                                               ALU.mult, ALU.add)
                tct = work.tile([H, HB], EWDT, tag=f"tct{z}", name=f"tct{z}")
                M = work.tile([H, HB], EWDT, tag=f"M{z}", name=f"M{z}")
                with tc.high_priority():
                    nc.scalar.activation(tct[:], S[:], AF.Tanh, scale=0.5)
                    # 2h = (th_o+1)*tct, split so the slow GpSimd engine
                    # absorbs the multiply (it cannot run TensorScalarPtr)
                    nc.gpsimd.tensor_mul(M[:], th_o, tct[:])
                    nxt = sz[z][(t + 1) % 2]
                    nc.vector.tensor_add(nxt[0:H, :], M[:], tct[:])
                hb = slice(z * HB, (z + 1) * HB)
                if t < T - 1:
                    nc.sync.dma_start(nxt[H:H + IN, :], x_d[t + 1, :, hb])
                elif t == T - 1:
                    nc.vector.memset(S[:], 0.0)
                else:
                    d = t - T
                    yp = pt[0:OUT, 3 * HB:4 * HB]
                    nc.tensor.matmul(yp, wy[:], nxt[:], start=True, stop=True)
                    yo = work.tile([OUT, HB], F32, tag=f"yo{z}", name=f"yo{z}")
                    nc.vector.tensor_copy(yo[:], yp)
                    nc.sync.dma_start(y_d[d, :, hb], yo[:])

            # Software pipeline: chain z's P1 interleaves with the other
            # chain's pending tail, anti-phasing the chains on the
            # in-order engines.
            pend = {}
            for t in range(T + DEC):
                for z in range(nh):
                    pt, th = emit_P1(z, t)
                    zo = (z + 1) % nh
                    if zo in pend:
                        emit_P2(**pend.pop(zo))
                    pend[z] = dict(z=z, t=t, pt=pt, th=th)
            for z in list(pend):
                emit_P2(**pend.pop(z))
    nc.finalize()
    return nc


def kernel(inputs, W_ih_enc, W_hh_enc, b_ih_enc, b_hh_enc,
           W_ih_dec, W_hh_dec, b_ih_dec, b_hh_dec, W_y, b_y,
           _trace=False, _perf_out=None):
    inputs = np.asarray(inputs, np.float32)
    f64 = np.float64
    encw = _pack_weights(np.asarray(W_hh_enc, f64), np.asarray(W_ih_enc, f64),
                         np.asarray(b_ih_enc, f64) + np.asarray(b_hh_enc, f64))
    Wihd = np.asarray(W_ih_dec, f64)
    Whhd = np.asarray(W_hh_dec, f64)
    bd = np.asarray(b_ih_dec, f64) + np.asarray(b_hh_dec, f64)
    Wyf = np.asarray(W_y, f64)
    byf = np.asarray(b_y, f64)
    dec0w = _pack_weights(Whhd, None, bd)
    decfw = _pack_weights(Whhd + Wihd @ Wyf, None, bd + Wihd @ byf)
    wyk = np.zeros((K, OUT), np.float32)
    wyk[0:H, :] = Wyf.T.astype(np.float32) * 0.5
    wyk[K - 1, :] = byf.astype(np.float32)
    ones = np.ones((1, BS), np.float32)

    nc = _build_program()

    bf = ml_dtypes.bfloat16
    encw, dec0w, decfw = encw.astype(bf), dec0w.astype(bf), decfw.astype(bf)
    wyk, ones = wyk.astype(bf), ones.astype(bf)
    in_maps = []
    for core in range(NCORES):
        xs = inputs[:, core * BS:(core + 1) * BS, :]         # [T, BS, IN]
        xt = np.ascontiguousarray(xs.transpose(0, 2, 1))     # [T, IN, BS]
        in_maps.append({"x": xt.astype(bf), "encw": encw, "dec0w": dec0w,
                        "decfw": decfw, "wy": wyk, "ones": ones})

    import time as _time
    res = run_bass_kernel_spmd(nc, in_maps, core_ids=list(range(NCORES)),
                               trace=_trace)
    if _perf_out is not None:
        walls = []
        for _ in range(6):
            t0 = _time.time()
            res = run_bass_kernel_spmd(nc, in_maps,
                                       core_ids=list(range(NCORES)),
                                       trace=_trace)
            walls.append(int((_time.time() - t0) * 1e9))
        _perf_out.update(exec_time_ns=res.exec_time_ns, walls_ns=walls,
                         trace=res.instructions_and_trace,
                         profile_json=res.profile_json)
    out = np.empty((DEC, B, OUT), np.float32)
    for core in range(NCORES):
        y = res.results[core]["y"]                           # [DEC, OUT, BS]
        out[:, core * BS:(core + 1) * BS, :] = y.transpose(0, 2, 1)
    return out


# revision 8
# speedup vs baseline: 1.2460x; 1.2040x over previous
"""Seq2seq LSTM (CoordinatePredictionModel) Trainium2 Bass kernel.

Model: 200-step LSTM encoder over [T=200, B=4096, IN=4], then 30-step
autoregressive LSTM decoder with output projection -> [30, B, OUT=4].

Sharding: pure data-parallel over batch. B=4096 -> 512 per core x 8 cores,
no collectives. Each core runs the full 230-step recurrence.

Layout ("hidden on partitions"): per-step state lives transposed in one
SBUF tile s = [K=105, HB]: rows 0..99 = (2h)^T, rows 100..103 = x^T
(encoder input; zero-weighted in decoder), row 104 = ones. Gate
pre-activations for chunk g come from one matmul
  psum[:, g*HB:(g+1)*HB] = W_g^T.T @ s       (K=105 contraction)
with W_g^T = [W_hh_g.T ; W_ih_g.T ; (b_ih+b_hh)_g] stacked on partitions.

All-tanh gates: sigmoid(x) = (tanh(x/2)+1)/2, with the 1/2 folded into
the packed i/f/o weight columns, so ONE scalar-engine tanh covers all
four gate chunks [100, 4*HB] per step (the scalar engine's per-
instruction overhead was the baseline bottleneck). States are stored
scaled: S = 2c, sh = 2h (the 1/2 folded into weight h-rows), making the
cell update three fused scalar_tensor_tensor ops:
  b  = (th_i + 1) * th_g          (DVE)      = 2 sig(i) tanh(g)
  aa = (th_f + 1) * S             (GpSimd)   = 4 sig(f) c
  S' = 0.5*aa + b                 (DVE)      = 2 c'
  tct = tanh(0.5 * S')            (ACT, free input scale) = tanh(c')
  sh' = (th_o + 1) * tct          (DVE)      = 2 h'

Decoder feedback y_prev = W_y h + b_y is folded into the recurrence:
  W_ih_dec @ y_prev + W_hh_dec @ h = (W_ih_dec W_y + W_hh_dec) @ h + W_ih_dec b_y
(valid from the second decoder step; the first uses y_prev = 0), so y is
only ever computed PSUM -> DRAM and never re-enters SBUF state.

Two batch half-chains (nh=2) are software-pipelined anti-phase so each
engine alternates between chains and dependency latency is hidden.

Only DMA may write non-32-aligned partition bases; all engine writes here
start at partition 0 (x and ones rows are DMA-written).
"""

import ml_dtypes
import numpy as np

import concourse.bass as bass
import concourse.mybir as mybir
from concourse import bacc
from concourse.tile import TileContext
from concourse.tile_rust import add_dep_helper
from concourse.bass_utils import run_bass_kernel_spmd

T, B, IN, OUT, H = 200, 4096, 4, 4, 100
DEC = 30
NCORES = 8
BS = B // NCORES          # 512 batch rows per core
K = H + IN + 1            # 105 = h + x + ones
F32 = mybir.dt.float32

# matmul operand dtype: bf16 streams 1 column/cycle on the PE (fp32/fp32r
# measured 4 cycles/col); PSUM accumulation stays fp32. End-to-end rms
# impact measured 8.7e-4 in a numpy bit-accurate simulation.
MDT = mybir.dt.bfloat16
EWDT = F32

ALU = mybir.AluOpType


# gate order: pytorch [i, f, g, o] -> ours [i, g, f, o] so the two gates
# needed first by the tail (i for b, g for b) are contiguous and their
# tanh can issue as soon as the first two matmuls finish.
_PERM = np.concatenate([np.arange(0, 100), np.arange(200, 300),
                        np.arange(100, 200), np.arange(300, 400)])


def _pack_weights(W_hh, W_ih, bias):
    """[K=105, 4H] stacked lhsT, gate order [i,g,f,o].

    i/f/o columns scaled by 1/2 (sigmoid-as-tanh), h-rows by a further
    1/2 (state stores 2h)."""
    cs = np.ones((4 * H,), np.float64)
    cs[0:H] = 0.5              # i
    cs[2 * H:4 * H] = 0.5      # f, o
    Wk = np.zeros((K, 4 * H), np.float64)
    Wk[0:H, :] = W_hh.T[:, _PERM] * cs * 0.5
    if W_ih is not None:
        Wk[H:H + W_ih.shape[1], :] = W_ih.T[:, _PERM] * cs
    Wk[K - 1, :] = bias[_PERM] * cs
    return Wk.astype(np.float32)


def _build_program(nh=2):
    """nh = number of independent batch half-chains interleaved per step."""
    nc = bacc.Bacc("TRN2", debug=False, num_devices=NCORES)

    x_d = nc.dram_tensor("x", (T, IN, BS), MDT, kind="ExternalInput").ap()
    encw_d = nc.dram_tensor("encw", (K, 4 * H), MDT, kind="ExternalInput").ap()
    dec0w_d = nc.dram_tensor("dec0w", (K, 4 * H), MDT, kind="ExternalInput").ap()
    decfw_d = nc.dram_tensor("decfw", (K, 4 * H), MDT, kind="ExternalInput").ap()
    wy_d = nc.dram_tensor("wy", (K, OUT), MDT, kind="ExternalInput").ap()
    ones_d = nc.dram_tensor("ones", (1, BS), MDT, kind="ExternalInput").ap()
    y_d = nc.dram_tensor("y", (DEC, OUT, BS), F32, kind="ExternalOutput").ap()

    AF = mybir.ActivationFunctionType
    HB = BS // nh

    with TileContext(nc) as tc:
        with (
            tc.tile_pool(name="const", bufs=1) as constp,
            tc.tile_pool(name="state", bufs=1) as statep,
            tc.tile_pool(name="work", bufs=2) as work,
            tc.tile_pool(name="psum", bufs=2, space="PSUM") as psump,
        ):
            encw = constp.tile([K, 4 * H], MDT, tag="encw")
            dec0w = constp.tile([K, 4 * H], MDT, tag="dec0w")
            decfw = constp.tile([K, 4 * H], MDT, tag="decfw")
            wy = constp.tile([K, OUT], MDT, tag="wy")
            nc.sync.dma_start(encw[:], encw_d[:])
            nc.sync.dma_start(dec0w[:], dec0w_d[:])
            nc.sync.dma_start(decfw[:], decfw_d[:])
            nc.sync.dma_start(wy[:], wy_d[:])

            sz = [[statep.tile([K, HB], MDT, tag=f"s{p}{z}", name=f"s{p}{z}")
                   for p in range(2)] for z in range(nh)]
            cz = [statep.tile([H, HB], EWDT, tag=f"c{z}", name=f"c{z}")
                  for z in range(nh)]

            for z in range(nh):
                hb = slice(z * HB, (z + 1) * HB)
                nc.gpsimd.memset(sz[z][0][0:H, :].bitcast(mybir.dt.uint16), 0)
                nc.gpsimd.memset(cz[z][:], 0.0)
                nc.sync.dma_start(sz[z][0][K - 1:K, :], ones_d[0:1, hb])
                nc.sync.dma_start(sz[z][1][K - 1:K, :], ones_d[0:1, hb])
                nc.sync.dma_start(sz[z][0][H:H + IN, :], x_d[0, :, hb])

            def emit_P1(z, t):
                """4 gate matmuls + split all-tanh (i,g first). Returns (pt, th)."""
                W = encw if t < T else (dec0w if t == T else decfw)
                prev = sz[z][t % 2]
                pt = psump.tile([H, 4 * HB], F32, tag=f"pt{z}", name=f"pt{z}")
                for g in range(4):
                    nc.tensor.matmul(pt[:, g * HB:(g + 1) * HB],
                                     W[:, g * H:(g + 1) * H], prev[:],
                                     start=True, stop=True)
                th = work.tile([H, 4 * HB], EWDT, tag=f"th{z}", name=f"th{z}")
                # [i,g] tanh only waits on the first two matmuls; [f,o]
                # hides behind the DVE tail that consumes th_i/th_g.
                nc.scalar.activation(th[:, 0:2 * HB], pt[:, 0:2 * HB], AF.Tanh)
                nc.scalar.activation(th[:, 2 * HB:4 * HB], pt[:, 2 * HB:4 * HB],
                                     AF.Tanh)
                return pt, th

            def emit_P2(z, t, pt, th):
                """Cell-state tail for chain z, step t."""
                S = cz[z]
                th_i = th[:, 0:HB]
                th_g = th[:, HB:2 * HB]
                th_f = th[:, 2 * HB:3 * HB]
                th_o = th[:, 3 * HB:4 * HB]
                b = work.tile([H, HB], EWDT, tag=f"b{z}", name=f"b{z}")
                nc.vector.scalar_tensor_tensor(b[:], th_i, 1.0, th_g,
                                               ALU.add, ALU.mult)
                aa = work.tile([H, HB], EWDT, tag=f"aa{z}", name=f"aa{z}")
                nc.vector.scalar_tensor_tensor(aa[:], th_f, 1.0, S[:],
                                               ALU.add, ALU.mult)
                nc.vector.scalar_tensor_tensor(S[:], aa[:], 0.5, b[:],
                                               ALU.mult, ALU.add)
                tct = work.tile([H, HB], EWDT, tag=f"tct{z}", name=f"tct{z}")
                with tc.high_priority():
                    nc.scalar.activation(tct[:], S[:], AF.Tanh, scale=0.5)
                    nxt = sz[z][(t + 1) % 2]
                    nc.vector.scalar_tensor_tensor(nxt[0:H, :], th_o, 1.0,
                                                   tct[:], ALU.add, ALU.mult)
                hb = slice(z * HB, (z + 1) * HB)
                if t < T - 1:
                    nc.sync.dma_start(nxt[H:H + IN, :], x_d[t + 1, :, hb])
                elif t == T - 1:
                    nc.vector.memset(S[:], 0.0)
                else:
                    d = t - T
                    yp = pt[0:OUT, 3 * HB:4 * HB]
                    nc.tensor.matmul(yp, wy[:], nxt[:], start=True, stop=True)
                    yo = work.tile([OUT, HB], F32, tag=f"yo{z}", name=f"yo{z}")
                    nc.vector.tensor_copy(yo[:], yp)
                    nc.sync.dma_start(y_d[d, :, hb], yo[:])

            # Software pipeline: chain z's P1 interleaves with the other
            # chain's pending tail, anti-phasing the chains on the
            # in-order engines.
            pend = {}
            for t in range(T + DEC):
                for z in range(nh):
                    pt, th = emit_P1(z, t)
                    zo = (z + 1) % nh
                    if zo in pend:
                        emit_P2(**pend.pop(zo))
                    pend[z] = dict(z=z, t=t, pt=pt, th=th)
            for z in list(pend):
                emit_P2(**pend.pop(z))
    nc.finalize()
    return nc


def kernel(inputs, W_ih_enc, W_hh_enc, b_ih_enc, b_hh_enc,
           W_ih_dec, W_hh_dec, b_ih_dec, b_hh_dec, W_y, b_y,
           _trace=False, _perf_out=None):
    inputs = np.asarray(inputs, np.float32)
    f64 = np.float64
    encw = _pack_weights(np.asarray(W_hh_enc, f64), np.asarray(W_ih_enc, f64),
                         np.asarray(b_ih_enc, f64) + np.asarray(b_hh_enc, f64))
    Wihd = np.asarray(W_ih_dec, f64)
    Whhd = np.asarray(W_hh_dec, f64)
    bd = np.asarray(b_ih_dec, f64) + np.asarray(b_hh_dec, f64)
    Wyf = np.asarray(W_y, f64)
    byf = np.asarray(b_y, f64)
    dec0w = _pack_weights(Whhd, None, bd)
    decfw = _pack_weights(Whhd + Wihd @ Wyf, None, bd + Wihd @ byf)
    wyk = np.zeros((K, OUT), np.float32)
    wyk[0:H, :] = Wyf.T.astype(np.float32) * 0.5
    wyk[K - 1, :] = byf.astype(np.float32)
    ones = np.ones((1, BS), np.float32)

    nc = _build_program()

    bf = ml_dtypes.bfloat16
    encw, dec0w, decfw = encw.astype(bf), dec0w.astype(bf), decfw.astype(bf)
    wyk, ones = wyk.astype(bf), ones.astype(bf)
    in_maps = []
    for core in range(NCORES):
        xs = inputs[:, core * BS:(core + 1) * BS, :]         # [T, BS, IN]
        xt = np.ascontiguousarray(xs.transpose(0, 2, 1))     # [T, IN, BS]
        in_maps.append({"x": xt.astype(bf), "encw": encw, "dec0w": dec0w,
                        "decfw": decfw, "wy": wyk, "ones": ones})

    import time as _time
    res = run_bass_kernel_spmd(nc, in_maps, core_ids=list(range(NCORES)),
                               trace=_trace)
    if _perf_out is not None:
        walls = []
        for _ in range(6):
            t0 = _time.time()
            res = run_bass_kernel_spmd(nc, in_maps,
                                       core_ids=list(range(NCORES)),
                                       trace=_trace)
            walls.append(int((_time.time() - t0) * 1e9))
        _perf_out.update(exec_time_ns=res.exec_time_ns, walls_ns=walls,
                         trace=res.instructions_and_trace,
                         profile_json=res.profile_json)
    out = np.empty((DEC, B, OUT), np.float32)
    for core in range(NCORES):
        y = res.results[core]["y"]                           # [DEC, OUT, BS]
        out[:, core * BS:(core + 1) * BS, :] = y.transpose(0, 2, 1)
    return out
